# revision 29
# baseline (speedup 1.0000x reference)
"""Trainium2 Bass kernel for nn_CONCATNet_7447473291796 (gnn_message_passing).

Strategy (pure data parallelism, batch sharded 16 per core across 8 cores):
  The reference only ever *uses* ~66 of the 4096 wafer rows per batch. The
  host gathers exactly those rows (plus the stage / next-stage / arm rows)
  while sharding the batch, and hands each core dense, pre-transposed bf16
  tiles with the embed dim on partitions:

    xcolA [128, 832]   w_cs | w_cw | stage rows pm 0..511 | arm-loc | next-stage
    xrowA [128, 576]   wafer rows pm 0..511 | arm-loc | arm-recipe
    xcolB/xrowB [128, 512]  pm columns 512..1023
    rfl   [1, 1472]    remain_prs per column + fused weight vectors

  The whole module is linear, so the robot-arm head is folded into
  host-precomputed fused weights (W_cs@W_rl, W_cw@W_rl, v_dyn@W_rl,
  colsum(W_rl) for the loc==P+1 ones row).  The device is then just
  12 matmuls in a transposed layout out[d_out, rows]:

    pmT  = W_cs.T @ xcol + W_cw.T @ xrow + v_dyn (x) rfl        (N=512, x2)
    armT = fused(W)s over the 64 arm columns + two rank-1 terms  (N=32)

  The rank-1 (v_dyn) matmuls come first in each PSUM group - they only
  need the tiny rfl load, so the PE starts before the big tiles land.
  Loads are spread over sync + scalar HWDGE and the gpsimd SWDGE queue.
  bf16 in/out with fp32 PSUM accumulation keeps rel err ~4e-3 (gate 2e-2).

All per-core variation lives in the DRAM inputs; the Bass program is
identical on every core.
"""

import numpy as np
import ml_dtypes

import concourse.bass as bass
import concourse.bacc as bacc
import concourse.mybir as mybir
import concourse.tile as tile
from concourse.bass_utils import run_bass_kernel_spmd

B, N, S, P, D = 128, 4096, 32, 64, 128
NORM = 300.0
NCORES = 8
BL = B // NCORES          # local batches per core = 16
R = BL * P                # pm columns per core = 1024
A = 2 * BL                # arm columns per core = 32
H = R // 2                # pm columns per tile = 512
WA = 2 * D                # w_cs|w_cw packed at the head of xcolA

# rfl layout offsets
RP_A, RP_B = 0, H
ARMR = R                  # 1024: remain_prs at the arm's loc
IND = R + A               # 1056: indicator loc == P+1
VDYN = R + 2 * A          # 1088: v_dyn
VDYN_RL = VDYN + D        # 1216: v_dyn @ W_rl
CSUM = VDYN_RL + D        # 1344: colsum(W_rl)
RFLW = CSUM + D           # 1472

F32 = mybir.dt.float32
BF16 = mybir.dt.bfloat16
BF = ml_dtypes.bfloat16

_prog_cache = None


def _build_program():
    """Hand-scheduled raw-bass program (no TileContext): explicit per-DMA
    semaphores, engines free-run with minimal waits."""
    nc = bacc.Bacc("TRN2", target_bir_lowering=False, debug=False)

    xcolA1_h = nc.declare_dram_parameter("xcolA1", [128, WA + H // 2], BF16,
                                         isOutput=False)
    xcolA2_h = nc.declare_dram_parameter("xcolA2", [128, H // 2 + 2 * A], BF16,
                                         isOutput=False)
    xcolB_h = nc.declare_dram_parameter("xcolB", [128, H], BF16, isOutput=False)
    xrowA_h = nc.declare_dram_parameter("xrowA", [128, H + 2 * A], BF16, isOutput=False)
    xrowB_h = nc.declare_dram_parameter("xrowB", [128, H], BF16, isOutput=False)
    wB_h = nc.declare_dram_parameter("wB", [128, 4, D], BF16, isOutput=False)
    rfl_h = nc.declare_dram_parameter("rfl", [1, RFLW], BF16, isOutput=False)

    out0a_h = nc.declare_dram_parameter("out0a", [128, H + A], BF16, isOutput=True)
    out1_h = nc.declare_dram_parameter("out1", [128, H], BF16, isOutput=True)

    from contextlib import ExitStack
    with ExitStack() as stack:
        ec = stack.enter_context
        rfl = ec(nc.sbuf_tensor([1, RFLW], BF16))
        xcolA1 = ec(nc.sbuf_tensor([128, WA + H // 2], BF16))
        xcolA2 = ec(nc.sbuf_tensor([128, H // 2 + 2 * A], BF16))
        xcolB = ec(nc.sbuf_tensor([128, H], BF16))
        xrowA = ec(nc.sbuf_tensor([128, H + 2 * A], BF16))
        xrowB = ec(nc.sbuf_tensor([128, H], BF16))
        wBsb = ec(nc.sbuf_tensor([128, 4, D], BF16))
        o0a = ec(nc.sbuf_tensor([128, H + A], BF16))
        o1 = ec(nc.sbuf_tensor([128, H], BF16))
        ps0 = ec(nc.psum_tensor([128, H], F32))
        ps1 = ec(nc.psum_tensor([128, H], F32))
        psr = ec(nc.psum_tensor([128, A], F32))
        s_rfl = ec(nc.semaphore("s_rfl"))
        s_xcA1 = ec(nc.semaphore("s_xcA1"))
        s_xcA2 = ec(nc.semaphore("s_xcA2"))
        s_xcB = ec(nc.semaphore("s_xcB"))
        s_xrA = ec(nc.semaphore("s_xrA"))
        s_xrB = ec(nc.semaphore("s_xrB"))
        s_wB = ec(nc.semaphore("s_wB"))
        s_st0 = ec(nc.semaphore("s_st0"))
        s_st1 = ec(nc.semaphore("s_st1"))
        t0 = ec(nc.semaphore("t0"))
        t1 = ec(nc.semaphore("t1"))
        t2 = ec(nc.semaphore("t2"))
        v0 = ec(nc.semaphore("v0"))
        v1 = ec(nc.semaphore("v1"))
        c0 = ec(nc.semaphore("c0"))
        block = ec(nc.Block())
        w_cs = xcolA1[:, 0:D]
        w_cw = xcolA1[:, D : 2 * D]
        w_rw = wBsb[:, 0, :]
        w_rn = wBsb[:, 1, :]
        w_fcs = wBsb[:, 2, :]    # W_cs @ W_rl
        w_fcw = wBsb[:, 3, :]    # W_cw @ W_rl
        v_dyn = rfl[:, VDYN : VDYN + D]
        v_dyn_rl = rfl[:, VDYN_RL : VDYN_RL + D]
        v_csum = rfl[:, CSUM : CSUM + D]
        xcA1 = xcolA1[:, WA : WA + H // 2]       # stage pm 0..255
        xcA2 = xcolA2[:, 0 : H // 2]             # stage pm 256..511
        xcA_loc = xcolA2[:, H // 2 : H // 2 + A]
        xcA_ns = xcolA2[:, H // 2 + A : H // 2 + 2 * A]

        @block.sync
        def _(sync):
            sync.dma_start(xcolA1[:], xcolA1_h[:]).then_inc(s_xcA1, 16)
            sync.dma_start(xcolA2[:], xcolA2_h[:]).then_inc(s_xcA2, 16)
            sync.wait_ge(v0, 1)
            sync.wait_ge(c0, 1)
            sync.dma_start(out0a_h[:], o0a[:]).then_inc(s_st0, 16)
            sync.wait_ge(s_st0, 16)

        @block.scalar
        def _(scalar):
            scalar.dma_start(rfl[:], rfl_h[:]).then_inc(s_rfl, 16)
            scalar.dma_start(xrowA[:], xrowA_h[:]).then_inc(s_xrA, 16)
            scalar.dma_start(wBsb[:], wB_h[:]).then_inc(s_wB, 16)
            scalar.wait_ge(t2, 1)
            scalar.copy(out=o0a[:, H : H + A], in_=psr[:]).then_inc(c0, 1)
            scalar.wait_ge(v1, 1)
            scalar.dma_start(out1_h[:], o1[:]).then_inc(s_st1, 16)
            scalar.wait_ge(s_st1, 16)

        @block.gpsimd
        def _(g):
            g.dma_start(xcolB[:], xcolB_h[:]).then_inc(s_xcB, 16)
            g.dma_start(xrowB[:], xrowB_h[:]).then_inc(s_xrB, 16)

        @block.tensor
        def _(t):
            t.wait_ge(s_rfl, 16)
            # rank-1 terms run inside the big tiles' DMA latency window
            t.matmul(ps0[:], lhsT=v_dyn, rhs=rfl[:, RP_A : RP_A + H],
                     start=True, stop=False)
            t.matmul(psr[:], lhsT=v_dyn_rl, rhs=rfl[:, ARMR : ARMR + A],
                     start=True, stop=False)
            t.matmul(psr[:], lhsT=v_csum, rhs=rfl[:, IND : IND + A],
                     start=False, stop=False)
            t.matmul(ps1[:], lhsT=v_dyn, rhs=rfl[:, RP_B : RP_B + H],
                     start=True, stop=False)
            t.wait_ge(s_xcA1, 16)
            t.matmul(ps0[:, 0 : H // 2], lhsT=w_cs, rhs=xcA1,
                     start=False, stop=False)
            t.wait_ge(s_xcA2, 16)
            t.matmul(ps0[:, H // 2 : H], lhsT=w_cs, rhs=xcA2,
                     start=False, stop=False)
            t.wait_ge(s_xrA, 16)
            t.matmul(ps0[:], lhsT=w_cw, rhs=xrowA[:, 0:H],
                     start=False, stop=True).then_inc(t0, 1)
            t.wait_ge(s_xcB, 16)
            t.matmul(ps1[:], lhsT=w_cs, rhs=xcolB[:], start=False, stop=False)
            t.wait_ge(s_xrB, 16)
            t.matmul(ps1[:], lhsT=w_cw, rhs=xrowB[:],
                     start=False, stop=True).then_inc(t1, 1)
            t.wait_ge(s_wB, 16)
            t.matmul(psr[:], lhsT=w_fcs, rhs=xcA_loc, start=False, stop=False)
            t.matmul(psr[:], lhsT=w_fcw, rhs=xrowA[:, H : H + A],
                     start=False, stop=False)
            t.matmul(psr[:], lhsT=w_rw, rhs=xrowA[:, H + A : H + 2 * A],
                     start=False, stop=False)
            t.matmul(psr[:], lhsT=w_rn, rhs=xcA_ns,
                     start=False, stop=True).then_inc(t2, 1)

        @block.vector
        def _(v):
            v.wait_ge(t0, 1)
            v.tensor_copy(out=o0a[:, 0:H], in_=ps0[:]).then_inc(v0, 1)
            v.wait_ge(t1, 1)
            v.tensor_copy(out=o1[:], in_=ps1[:]).then_inc(v1, 1)

    nc.compile()
    return nc


def _get_program():
    global _prog_cache
    if _prog_cache is None:
        _prog_cache = _build_program()
    return _prog_cache


def make_in_maps(inputs):
    inputs = {k: np.asarray(v) for k, v in inputs.items()}
    er = inputs["encoded_row"].astype(np.float32)          # [B, N, D]
    ec = inputs["encoded_col"].astype(np.float32)          # [B, S, D]
    clock = inputs["clock"].astype(np.float32)             # [B, 1]
    lpet = inputs["loc_process_end_time"].astype(np.float32)  # [B, P]
    W_dyn = inputs["W_dyn"].astype(np.float32)
    W_concat = inputs["W_concat"].astype(np.float32)
    W_robot = inputs["W_robot"].astype(np.float32)
    lhw = inputs["loc_hold_wafer"].astype(np.int64)        # [B, P]
    lst = inputs["loc_stage"].astype(np.int64)             # [B, P]
    loc = np.concatenate([inputs["robot_arm1_loc"], inputs["robot_arm2_loc"]],
                         axis=1).astype(np.int64)          # [B, 2]
    rec = np.concatenate([inputs["arm1_recipe"], inputs["arm2_recipe"]],
                         axis=1).astype(np.int64)          # [B, 2]
    nst = np.concatenate([inputs["arm1_next_stage"], inputs["arm2_next_stage"]],
                         axis=1).astype(np.int64)          # [B, 2]

    # pm ingredients, full batch
    rp = np.maximum(lpet - clock, 0.0) / NORM              # [B, P]
    wafer = np.where(
        (lhw >= 0)[:, :, None],
        np.take_along_axis(er, np.clip(lhw, 0, N - 1)[:, :, None], axis=1),
        0.0,
    )                                                      # [B, P, D]
    stage = np.take_along_axis(ec, (lst - 1)[:, :, None], axis=1)  # [B, P, D]

    # arm ingredients
    locv = (loc >= 1) & (loc <= P)                         # [B, 2]
    pidx = np.clip(loc - 1, 0, P - 1)
    armw = np.where(locv[:, :, None],
                    np.take_along_axis(wafer, pidx[:, :, None], axis=1), 0.0)
    arms = np.where(locv[:, :, None],
                    np.take_along_axis(stage, pidx[:, :, None], axis=1), 0.0)
    armr = np.where(locv, np.take_along_axis(rp, pidx, axis=1), 0.0)  # [B, 2]
    ind = (loc == P + 1).astype(np.float32)                # [B, 2]
    rrow = np.where(
        (rec >= 0)[:, :, None],
        np.take_along_axis(er, np.clip(rec, 0, N - 1)[:, :, None], axis=1),
        0.0,
    )                                                      # [B, 2, D]
    nsv = (nst >= 1) & (nst <= S)
    nrow = np.where(
        nsv[:, :, None],
        np.take_along_axis(ec, np.clip(nst - 1, 0, S - 1)[:, :, None], axis=1),
        0.0,
    )                                                      # [B, 2, D]

    # weights (+ fused arm head: the module is linear in pm_emb)
    W_cs, W_cw, W_cd = W_concat[0:D], W_concat[D : 2 * D], W_concat[2 * D : 3 * D]
    W_rl, W_rw, W_rn = W_robot[0:D], W_robot[D : 2 * D], W_robot[2 * D : 3 * D]
    v_dyn = (W_dyn[0:1] @ W_cd).reshape(D)
    wA = np.concatenate([W_cs, W_cw], axis=1).astype(BF)   # [128, 2D]
    wB = np.ascontiguousarray(
        np.stack([W_rw, W_rn, W_cs @ W_rl, W_cw @ W_rl], axis=1)
    ).astype(BF)                                           # [128, 4, D]
    v_dyn_rl = v_dyn @ W_rl                                # [D]
    v_csum = W_rl.sum(axis=0)                              # [D]

    in_maps = []
    for c in range(NCORES):
        bs = slice(c * BL, (c + 1) * BL)
        xrow = np.concatenate(
            [wafer[bs].reshape(R, D), armw[bs].reshape(A, D),
             rrow[bs].reshape(A, D)], axis=0).T            # [D, R+2A]
        xcol = np.concatenate(
            [stage[bs].reshape(R, D), arms[bs].reshape(A, D),
             nrow[bs].reshape(A, D)], axis=0).T
        xrow = np.ascontiguousarray(xrow).astype(BF)
        xcol = np.ascontiguousarray(xcol).astype(BF)
        rfl = np.concatenate(
            [rp[bs].reshape(R), armr[bs].reshape(A), ind[bs].reshape(A),
             v_dyn, v_dyn_rl, v_csum]).reshape(1, RFLW).astype(BF)
        in_maps.append({
            "xcolA1": np.ascontiguousarray(np.concatenate(
                [wA, xcol[:, 0 : H // 2]], axis=1)),
            "xcolA2": np.ascontiguousarray(np.concatenate(
                [xcol[:, H // 2 : H], xcol[:, R : R + 2 * A]], axis=1)),
            "xcolB": np.ascontiguousarray(xcol[:, H:R]),
            "xrowA": np.ascontiguousarray(
                np.concatenate([xrow[:, 0:H], xrow[:, R : R + 2 * A]], axis=1)),
            "xrowB": np.ascontiguousarray(xrow[:, H:R]),
            "wB": wB,
            "rfl": rfl,
        })
    return in_maps


def assemble_output(res):
    out = np.empty((B, P + 2, D), np.float32)
    for c in range(NCORES):
        bs = slice(c * BL, (c + 1) * BL)
        o0a = np.asarray(res[c]["out0a"])
        pmT = np.concatenate(
            [o0a[:, 0:H], np.asarray(res[c]["out1"])], axis=1
        ).astype(np.float32)                               # [D, R]
        out[bs, 0:P, :] = pmT.T.reshape(BL, P, D)
        armT = o0a[:, H : H + A].astype(np.float32)        # [D, A]
        out[bs, P:, :] = armT.T.reshape(BL, 2, D)
    return out


def kernel(**inputs):
    in_maps = make_in_maps(inputs)
    nc = _get_program()
    res = run_bass_kernel_spmd(nc, in_maps, list(range(NCORES))).results
    return assemble_output(res)
